# revision 8
# baseline (speedup 1.0000x reference)
"""Trainium2 Bass kernel for nn_CustomLayerMKM: y = x @ (sum_k kron(Bk, Ak)).T + bias.

Exploits the Kronecker structure instead of materializing the dense 4096x4096
weight: kron(Bk,Ak) = kron(Bk,I) @ kron(I,Ak), so each factor is two cheap
matmuls (~9x fewer FLOPs than dense).

Sharding: data-parallel over B across 8 cores (512 rows each); the small
Kronecker factors are replicated. No collectives.

Per-core pipeline (b processed in 2 halves of 256 = 2 j-slices of 128):
  1. load x slice, PE-transpose -> xT (i on partitions), bf16
  2. stage 1: per 128-wide i-block t: U_k = xT_block.T @ patA_k  (PSUM->SBUF)
     U_k free index fidx = u*128 + w*f1 + t*G + g   (u = q mod 32)
  3. corner-turn: PE-transpose 128-col chunks of U_k -> V' chunks
  4. stage 2: per output group u: one PSUM tile accumulates all 3 factors
     (lhsT = V'_k chunk slice, rhs = patB_k) plus a K=1 bias matmul, then a
     single strided eviction writes y columns o = c*32 + u.
patA/patB are permuted copies of the (runtime) factor weights built on device
with small SBUF->SBUF DMAs.
"""

from contextlib import ExitStack

import numpy as np

P = 128
B_FULL, I_DIM, O_DIM = 4096, 4096, 4096
N_CORES = 8
B_SHARD = B_FULL // N_CORES          # 512 rows per core
N_HALF = 2                           # b-shard processed in halves of 256
JPH = 2                              # 128-row j-slices per half
# (m, f1) per factor: A_k is (m, m) applied on the I-minor side, B_k is (f1, f1)
FACTOR_DIMS = [(64, 64), (128, 32), (32, 128)]
N_FAC = 3
TB = I_DIM // P                      # 32 i-blocks
UG = 32                              # output groups u = q mod 32
MM_DTYPE = "bfloat16"


def build_nc():
    import concourse.bass as bass
    import concourse.mybir as mybir
    import concourse.tile as tile
    from concourse import bacc
    from concourse.masks import make_identity

    MM_DT = getattr(mybir.dt, MM_DTYPE)
    F32 = mybir.dt.float32
    ts = bass.ts

    nc = bacc.Bacc("TRN2", target_bir_lowering=False, debug=False,
                   num_devices=N_CORES)

    x_ext = nc.dram_tensor("x", [B_SHARD, I_DIM], F32, kind="ExternalInput").ap()
    wa_ext, wb_ext = [], []
    for k, (m, f1) in enumerate(FACTOR_DIMS):
        wa_ext.append(nc.dram_tensor(f"w{k}a", [m, m], F32,
                                     kind="ExternalInput").ap())
        wb_ext.append(nc.dram_tensor(f"w{k}b", [f1, f1], F32,
                                     kind="ExternalInput").ap())
    bias_ext = nc.dram_tensor("bias", [1, O_DIM], F32, kind="ExternalInput").ap()
    y_ext = nc.dram_tensor("y", [B_SHARD, O_DIM], F32, kind="ExternalOutput").ap()

    with tile.TileContext(nc) as tc, ExitStack() as ctx:
        const = ctx.enter_context(tc.tile_pool(name="const", bufs=1))
        ps = ctx.enter_context(tc.tile_pool(name="ps", bufs=8, space="PSUM"))
        xin = ctx.enter_context(tc.tile_pool(name="xin", bufs=2))
        xtp = ctx.enter_context(tc.tile_pool(name="xtp", bufs=1))
        upool = ctx.enter_context(tc.tile_pool(name="upool", bufs=1))
        vpool = ctx.enter_context(tc.tile_pool(name="vpool", bufs=12))
        ypool = ctx.enter_context(tc.tile_pool(name="ypool", bufs=1))

        # ---------- constants ----------
        ident_f32 = const.tile([P, P], F32, tag="idf")
        make_identity(nc, ident_f32)
        ident_mm = const.tile([P, P], MM_DT, tag="idm")
        make_identity(nc, ident_mm)

        ones_mm = const.tile([1, P], MM_DT, tag="ones")
        nc.gpsimd.memset(ones_mm[:], 1.0)

        bias_sb = const.tile([1, O_DIM], F32, tag="biasf")
        nc.sync.dma_start(bias_sb[:], bias_ext[:])
        bias_mm = const.tile([1, O_DIM], MM_DT, tag="biasm")
        nc.vector.tensor_copy(bias_mm[:], bias_sb[:])
        bias_v = bias_mm.rearrange("p (c u) -> p u c", c=P, u=UG)

        patA, patB = [], []
        for k, (m, f1) in enumerate(FACTOR_DIMS):
            G, H = P // m, P // f1
            wa_sb = const.tile([m, m], F32, tag=f"wa{k}")
            nc.sync.dma_start(wa_sb[:], wa_ext[k][:])
            wb_sb = const.tile([f1, f1], F32, tag=f"wb{k}")
            nc.sync.dma_start(wb_sb[:], wb_ext[k][:])

            waT_ps = ps.tile([P, 512], F32, tag="ps")
            nc.tensor.transpose(waT_ps[:m, :m], wa_sb[:], ident_f32[:m, :m])
            waT = const.tile([m, m], MM_DT, tag=f"waT{k}")
            nc.vector.tensor_copy(waT[:], waT_ps[:m, :m])

            wbT_ps = ps.tile([P, 512], F32, tag="ps")
            nc.tensor.transpose(wbT_ps[:f1, :f1], wb_sb[:], ident_f32[:f1, :f1])
            wbT = const.tile([f1, f1], MM_DT, tag=f"wbT{k}")
            nc.vector.tensor_copy(wbT[:], wbT_ps[:f1, :f1])

            # patA[g*m + s, u*H*G + w*G + g] = A[u + 32*w, s]
            pa = const.tile([P, P], MM_DT, tag=f"patA{k}")
            nc.gpsimd.memset(pa[:], 0.0)
            src_a = waT.rearrange("p (w u) -> p u w", w=H, u=32)  # [s, u, w]
            dst_a = pa.rearrange("p (u w g) -> p g u w", u=32, w=H, g=G)
            for g in range(G):
                for w in range(H):
                    nc.sync.dma_start(dst_a[g * m:(g + 1) * m, g, :, w],
                                      src_a[:, :, w])

            # patB[wp*f1 + r, pp*H + wp] = B[pp, r]
            pb = const.tile([P, P], MM_DT, tag=f"patB{k}")
            nc.gpsimd.memset(pb[:], 0.0)
            dst_b = pb.rearrange("p (pp wp) -> p wp pp", pp=P // H, wp=H)
            for wp in range(H):
                nc.sync.dma_start(dst_b[wp * f1:(wp + 1) * f1, wp], wbT[:, :])
            patA.append(pa)
            patB.append(pb)

        # ---------- evictions alternate DVE / ACT ----------
        n_ev = [0]

        def evict(dst, src):
            if n_ev[0] % 2 == 0:
                nc.vector.tensor_copy(dst, src)
            else:
                nc.scalar.copy(dst, src)
            n_ev[0] += 1

        # ---------- main loop over b-halves ----------
        for bh in range(N_HALF):
            # xT[s, t, jl*128 + b]  (bf16)
            xT = xtp.tile([P, TB, JPH * P], MM_DT, tag="xT")
            for jl in range(JPH):
                x_sb = xin.tile([P, I_DIM], F32, tag="x")
                row0 = bh * (JPH * P) + jl * P
                nc.sync.dma_start(x_sb[:], x_ext[row0:row0 + P, :])
                for tp in range(TB // 4):
                    xt_ps = ps.tile([P, 512], F32, tag="ps")
                    for tl in range(4):
                        nc.tensor.transpose(
                            xt_ps[:, ts(tl, P)],
                            x_sb[:, ts(4 * tp + tl, P)],
                            ident_f32[:],
                        )
                    dst = xT[:, 4 * tp:4 * tp + 4, ts(jl, P)]
                    src = xt_ps.rearrange("p (tl c) -> p tl c", tl=4, c=P)
                    evict(dst, src)

            # stage 1: U_k[p, jl, fidx]
            U = [upool.tile([P, JPH, I_DIM], MM_DT, tag=f"U{k}", name=f"U{k}")
                 for k in range(N_FAC)]
            for jl in range(JPH):
                for T in range(TB // 4):
                    s1 = [ps.tile([P, 512], F32, tag="ps", name=f"s1_{jl}_{T}_{kk}")
                          for kk in range(N_FAC)]
                    for tl in range(4):
                        lhsT = xT[:, 4 * T + tl, ts(jl, P)]
                        for k in range(N_FAC):
                            nc.tensor.matmul(s1[k][:, ts(tl, P)], lhsT,
                                             patA[k][:], start=True, stop=True)
                    # evictions: src col c = u*4 + w*G + g  within each tl
                    uj0 = U[0][:, jl, :].rearrange(
                        "p (u w t2 tl g) -> p w u tl g t2",
                        u=32, w=2, t2=8, tl=4, g=2)
                    s0 = s1[0].rearrange("p (tl u w g) -> p w u tl g",
                                         tl=4, u=32, w=2, g=2)
                    for w in range(2):
                        evict(uj0[:, w, :, :, :, T], s0[:, w])
                    uj1 = U[1][:, jl, :].rearrange(
                        "p (u w t2 tl) -> p w u tl t2", u=32, w=4, t2=8, tl=4)
                    s_1 = s1[1].rearrange("p (tl u w) -> p w u tl",
                                          tl=4, u=32, w=4)
                    evict(uj1[:, :, :, :, T], s_1[:, :])
                    uj2 = U[2][:, jl, :].rearrange(
                        "p (u t2 tl g) -> p u tl g t2", u=32, t2=8, tl=4, g=4)
                    s_2 = s1[2].rearrange("p (tl u g) -> p u tl g",
                                          tl=4, u=32, g=4)
                    evict(uj2[:, :, :, :, T], s_2[:, :])

            # corner-turn + stage 2, per group of 4 u's
            yhalves = [ypool.tile([P, O_DIM], F32, tag=f"yh{jl}", name=f"yh{bh}_{jl}")
                       for jl in range(JPH)]
            for Ug4 in range(UG // 4):
                vch = [[None] * N_FAC for _ in range(2)]
                for up in range(2):
                    for k in range(N_FAC):
                        vt_ps = ps.tile([P, 512], MM_DT, tag="ps")
                        for ul2 in range(2):
                            u = Ug4 * 4 + up * 2 + ul2
                            for jl in range(JPH):
                                nc.tensor.transpose(
                                    vt_ps[:, ts(ul2 * 2 + jl, P)],
                                    U[k][:, jl, ts(u, P)],
                                    ident_mm[:],
                                )
                        vt = vpool.tile([P, 2, JPH * P], MM_DT, tag="vch")
                        evict(vt[:],
                              vt_ps.rearrange("p (a b) -> p a b", a=2,
                                              b=JPH * P))
                        vch[up][k] = vt
                for jl in range(JPH):
                    y_ps = ps.tile([P, 512], F32, tag="ps")
                    for ul in range(4):
                        u = Ug4 * 4 + ul
                        for k in range(N_FAC):
                            nc.tensor.matmul(
                                y_ps[:, ts(ul, P)],
                                vch[ul // 2][k][:, ul % 2, ts(jl, P)],
                                patB[k][:],
                                start=(k == 0), stop=False)
                        nc.tensor.matmul(y_ps[:, ts(ul, P)], ones_mm[:],
                                         bias_v[:, u, :], start=False,
                                         stop=True)
                    dst = yhalves[jl].rearrange("p (c u) -> p u c", c=P,
                                           u=UG)[:, Ug4 * 4:Ug4 * 4 + 4, :]
                    evict(dst, y_ps.rearrange("p (ul c) -> p ul c", ul=4, c=P))

            for jl in range(JPH):
                row0 = bh * (JPH * P) + jl * P
                nc.sync.dma_start(y_ext[row0:row0 + P, :], yhalves[jl][:])

    nc.compile()
    return nc


_NC_CACHE = {}


def kernel(**inputs):
    """Full-input entry point: shards over B, runs 8-core SPMD, gathers."""
    from concourse.bass_utils import run_bass_kernel_spmd

    x = np.ascontiguousarray(inputs["input_BI"], dtype=np.float32)
    bias = np.ascontiguousarray(inputs["bias_O"], dtype=np.float32)
    in_maps = []
    for c in range(N_CORES):
        im = {"x": x[c * B_SHARD:(c + 1) * B_SHARD],
              "bias": bias.reshape(1, O_DIM)}
        for k, (na, nb) in enumerate(
                [("w0a", "w0b"), ("w1a", "w1b"), ("w2a", "w2b")]):
            im[f"w{k}a"] = np.ascontiguousarray(inputs[na], dtype=np.float32)
            im[f"w{k}b"] = np.ascontiguousarray(inputs[nb], dtype=np.float32)
        in_maps.append(im)

    if "nc" not in _NC_CACHE:
        _NC_CACHE["nc"] = build_nc()
    res = run_bass_kernel_spmd(_NC_CACHE["nc"], in_maps,
                               core_ids=list(range(N_CORES)))
    return np.concatenate([r["y"] for r in res.results], axis=0)


# revision 13
# speedup vs baseline: 1.2568x; 1.2568x over previous
"""Trainium2 Bass kernel for nn_CustomLayerMKM: y = x @ (sum_k kron(Bk, Ak)).T + bias.

Exploits the Kronecker structure instead of materializing the dense 4096x4096
weight: kron(Bk,Ak) = kron(Bk,I) @ kron(I,Ak), so each factor costs two cheap
matmul stages (~9x fewer FLOPs than dense).

Sharding: data-parallel over B across 8 cores (512 rows each); the small
Kronecker factors are replicated. No collectives.

Host prep (cheap, O(B*I) element moves): x is pre-transposed + cast to bf16
per core; the 128x128 "pattern" matrices (permuted copies of the factor
weights) are built in numpy and passed as inputs.

Per-core device pipeline (b in 2 halves of 256 = 2 j-slices of 128):
  stage 1: per 128-wide i-block t: U_k = xT_block.T @ patA_k   (PE, N=128)
           U_k free index fidx = u*128 + w*f1 + t*G + g  (u = q mod 32)
  corner-turn: V_k = U_k.T via DMA-xbar transpose (bf16, 1 DMA per (k,jl))
  stage 2: per output group u: one PSUM tile accumulates all 3 factors
           (lhsT = V_k[:, u, :], rhs = patB_k) + a K=1 bias matmul, then one
           strided eviction writes y columns o = c*32 + u.
"""

from contextlib import ExitStack

import numpy as np

P = 128
B_FULL, I_DIM, O_DIM = 4096, 4096, 4096
N_CORES = 8
B_SHARD = B_FULL // N_CORES          # 512 rows per core
N_HALF = 2                           # b-shard processed in halves of 256
JPH = 2                              # 128-row j-slices per half
FACTOR_DIMS = [(64, 64), (128, 32), (32, 128)]   # (m, f1) per factor
N_FAC = 3
TB = I_DIM // P                      # 32 i-blocks
UG = 32                              # output groups u = q mod 32
MM_DTYPE = "bfloat16"


def build_nc(debug_dump=False):
    import concourse.bass as bass
    import concourse.mybir as mybir
    import concourse.tile as tile
    from concourse import bacc

    MM_DT = getattr(mybir.dt, MM_DTYPE)
    F32 = mybir.dt.float32
    ts = bass.ts

    nc = bacc.Bacc("TRN2", target_bir_lowering=False, debug=False,
                   num_devices=N_CORES)

    xT_ext = nc.dram_tensor("xT", [I_DIM, B_SHARD], MM_DT,
                            kind="ExternalInput").ap()
    pat_ext = {}
    for k in range(N_FAC):
        for nm in ("patA", "patB"):
            pat_ext[f"{nm}{k}"] = nc.dram_tensor(
                f"{nm}{k}", [P, P], MM_DT, kind="ExternalInput").ap()
    bias_ext = nc.dram_tensor("bias", [1, O_DIM], MM_DT,
                              kind="ExternalInput").ap()
    y_ext = nc.dram_tensor("y", [B_SHARD, O_DIM], F32,
                           kind="ExternalOutput").ap()
    dbg_ext = {}
    if debug_dump:
        for k in range(N_FAC):
            dbg_ext[f"Udbg{k}"] = nc.dram_tensor(
                f"Udbg{k}", [P, JPH * I_DIM], MM_DT,
                kind="ExternalOutput").ap()
            for jl in range(JPH):
                dbg_ext[f"Vdbg{k}_{jl}"] = nc.dram_tensor(
                    f"Vdbg{k}_{jl}", [P, TB * P], MM_DT,
                    kind="ExternalOutput").ap()

    with tile.TileContext(nc) as tc, ExitStack() as ctx:
        const = ctx.enter_context(tc.tile_pool(name="const", bufs=1))
        ps = ctx.enter_context(tc.tile_pool(name="ps", bufs=8, space="PSUM"))
        xtp = ctx.enter_context(tc.tile_pool(name="xtp", bufs=1))
        upool = ctx.enter_context(tc.tile_pool(name="upool", bufs=1))
        vpool = ctx.enter_context(tc.tile_pool(name="vpool", bufs=1))
        ypool = ctx.enter_context(tc.tile_pool(name="ypool", bufs=1))

        # ---------- constants ----------
        ones_mm = const.tile([1, P], MM_DT, tag="ones")
        nc.gpsimd.memset(ones_mm[:], 1.0)
        bias_mm = const.tile([1, O_DIM], MM_DT, tag="biasm")
        nc.sync.dma_start(bias_mm[:], bias_ext[:])
        # column o = c*32 + u  ->  view [u][c]
        bias_v = bias_mm.rearrange("p (c u) -> p u c", c=P, u=UG)

        patA, patB = [], []
        for k in range(N_FAC):
            pa = const.tile([P, P], MM_DT, tag=f"patA{k}")
            nc.sync.dma_start(pa[:], pat_ext[f"patA{k}"][:])
            pb = const.tile([P, P], MM_DT, tag=f"patB{k}")
            nc.sync.dma_start(pb[:], pat_ext[f"patB{k}"][:])
            patA.append(pa)
            patB.append(pb)

        xT_sb = xtp.tile([P, TB, B_SHARD], MM_DT, tag="xT")
        nc.sync.dma_start(xT_sb[:],
                          xT_ext.rearrange("(t p) b -> p t b", p=P, t=TB))

        # ---------- evictions alternate DVE / ACT ----------
        n_ev = [0]

        def evict(dst, src):
            if n_ev[0] % 2 == 0:
                nc.vector.tensor_copy(dst, src)
            else:
                nc.scalar.copy(dst, src)
            n_ev[0] += 1

        for bh in range(N_HALF):
            b0 = bh * (JPH * P)
            # ---- stage 1 ----
            U = [upool.tile([P, JPH, I_DIM], MM_DT, tag=f"U{k}", name=f"U{k}")
                 for k in range(N_FAC)]
            for jl in range(JPH):
                for T in range(TB // 4):
                    s1 = [ps.tile([P, 512], F32, tag="ps",
                                  name=f"s1_{bh}_{jl}_{T}_{kk}")
                          for kk in range(N_FAC)]
                    for tl in range(4):
                        lhsT = xT_sb[:, 4 * T + tl,
                                     b0 + jl * P:b0 + (jl + 1) * P]
                        for k in range(N_FAC):
                            nc.tensor.matmul(s1[k][:, ts(tl, P)], lhsT,
                                             patA[k][:], start=True, stop=True)
                    # src col c = u*4 + w*G + g within each tl-region
                    uj0 = U[0][:, jl, :].rearrange(
                        "p (u w t2 tl g) -> p w u tl g t2",
                        u=32, w=2, t2=8, tl=4, g=2)
                    s0 = s1[0].rearrange("p (tl u w g) -> p w u tl g",
                                         tl=4, u=32, w=2, g=2)
                    for w in range(2):
                        evict(uj0[:, w, :, :, :, T], s0[:, w])
                    uj1 = U[1][:, jl, :].rearrange(
                        "p (u w t2 tl) -> p w u tl t2", u=32, w=4, t2=8, tl=4)
                    s_1 = s1[1].rearrange("p (tl u w) -> p w u tl",
                                          tl=4, u=32, w=4)
                    evict(uj1[:, :, :, :, T], s_1[:, :])
                    uj2 = U[2][:, jl, :].rearrange(
                        "p (u t2 tl g) -> p u tl g t2", u=32, t2=8, tl=4, g=4)
                    s_2 = s1[2].rearrange("p (tl u g) -> p u tl g",
                                          tl=4, u=32, g=4)
                    evict(uj2[:, :, :, :, T], s_2[:, :])

            # ---- corner-turn via DMA-xbar transpose ----
            V = [[vpool.tile([P, TB, P], MM_DT, tag=f"V{k}_{jl}",
                             name=f"V{bh}_{k}_{jl}")
                  for jl in range(JPH)] for k in range(N_FAC)]
            for k in range(N_FAC):
                for jl in range(JPH):
                    nc.sync.dma_start_transpose(V[k][jl][:], U[k][:, jl, :])
            if debug_dump and bh == 0:
                for k in range(N_FAC):
                    nc.sync.dma_start(dbg_ext[f"Udbg{k}"][:],
                                      U[k].rearrange("p a b -> p (a b)"))
                    for jl in range(JPH):
                        nc.sync.dma_start(
                            dbg_ext[f"Vdbg{k}_{jl}"][:],
                            V[k][jl].rearrange("p a b -> p (a b)"))

            # ---- stage 2 ----
            yhalves = [ypool.tile([P, O_DIM], F32, tag=f"yh{jl}",
                                  name=f"yh{bh}_{jl}")
                       for jl in range(JPH)]
            for Ug4 in range(UG // 4):
                for jl in range(JPH):
                    y_ps = ps.tile([P, 512], F32, tag="ps",
                                   name=f"yps{bh}_{Ug4}_{jl}")
                    for ul in range(4):
                        u = Ug4 * 4 + ul
                        for k in range(N_FAC):
                            nc.tensor.matmul(
                                y_ps[:, ts(ul, P)],
                                V[k][jl][:, u, :],
                                patB[k][:],
                                start=(k == 0), stop=False)
                        nc.tensor.matmul(
                            y_ps[:, ts(ul, P)],
                            ones_mm[:],
                            bias_v[:, u, :],
                            start=False, stop=True)
                    dst = yhalves[jl].rearrange(
                        "p (c u) -> p u c", c=P, u=UG)[:, Ug4 * 4:Ug4 * 4 + 4, :]
                    evict(dst, y_ps.rearrange("p (ul c) -> p ul c", ul=4, c=P))

            for jl in range(JPH):
                row0 = b0 + jl * P
                nc.sync.dma_start(y_ext[row0:row0 + P, :], yhalves[jl][:])

    nc.compile()
    return nc


_NC_CACHE = {}


def prep_inputs(inputs):
    """Host preprocessing: per-core bf16 xT shards + pattern matrices."""
    import ml_dtypes

    bf16 = ml_dtypes.bfloat16
    x = np.asarray(inputs["input_BI"], dtype=np.float32)
    As = [np.asarray(inputs[n], dtype=np.float32) for n in ("w0a", "w1a", "w2a")]
    Bs = [np.asarray(inputs[n], dtype=np.float32) for n in ("w0b", "w1b", "w2b")]
    bias = np.asarray(inputs["bias_O"], dtype=np.float32)

    common = {"bias": np.ascontiguousarray(
        bias.reshape(1, O_DIM).astype(bf16))}
    for k, ((m, f1), A, Bk) in enumerate(zip(FACTOR_DIMS, As, Bs)):
        G, H = P // m, P // f1
        pa = np.zeros((P, P), np.float32)
        q_uw = np.arange(32)[:, None] + 32 * np.arange(H)[None, :]
        cols = (np.arange(32)[:, None] * H * G + np.arange(H)[None, :] * G)
        for g in range(G):
            pa[g * m:(g + 1) * m, (cols + g).ravel()] = A[q_uw.ravel(), :].T
        pb = np.zeros((P, P), np.float32)
        f2 = Bk.shape[0]
        for wp in range(H):
            pb[wp * f1:(wp + 1) * f1, np.arange(f2) * H + wp] = Bk.T
        common[f"patA{k}"] = np.ascontiguousarray(pa.astype(bf16))
        common[f"patB{k}"] = np.ascontiguousarray(pb.astype(bf16))

    in_maps = []
    for c in range(N_CORES):
        im = dict(common)
        im["xT"] = np.ascontiguousarray(
            x[c * B_SHARD:(c + 1) * B_SHARD].T.astype(bf16))
        in_maps.append(im)
    return in_maps


def kernel(**inputs):
    """Full-input entry point: shards over B, runs 8-core SPMD, gathers."""
    from concourse.bass_utils import run_bass_kernel_spmd

    in_maps = prep_inputs(inputs)
    if "nc" not in _NC_CACHE:
        _NC_CACHE["nc"] = build_nc()
    res = run_bass_kernel_spmd(_NC_CACHE["nc"], in_maps,
                               core_ids=list(range(N_CORES)))
    return np.concatenate([r["y"] for r in res.results], axis=0)
